# revision 1
# baseline (speedup 1.0000x reference)
"""Form-A GNN attention kernel (final).

out = (0.1*(deg/Z)*attm + 0.9*adj) @ in2, attm = adj*exp(lrelu(e1+e2)-8).

Structure (per core: 1024 rows, two 512-row PSUM groups):
- attm/adjT are the matmul STATIONARY operands ([128j,128row] slices); the
  moving operand is in2 augmented with a ones column (257 cols), so Z and
  deg fall out free as accumulator column 256 and the output lands in
  natural [row, d] layout. 1024 MMs x 257 cols ~= the f16 PE roofline.
- o1 (attention) matmuls run one 8-jc group BEHIND o2 (adjacency) matmuls:
  their attm stationaries are ready a full group period early, so the PE
  never idles and the HAM clock gate stays at 2.4 GHz.
- attm is produced on three engines in parallel: ScalarE Prelu->Exp for
  3-4 tiles/group (mask-mul on GpSimd or VectorE), VectorE tensor_scalar
  (u1*v1, u2*v2 from host-precomputed exp tables, exp(max)=max(exp) trick)
  + batched quad max/mask tensor_tensor for the rest.
- PSUM is evacuated with two big ScalarE copies right after the last MMs
  (banks free immediately); the deg/Z combine runs from SBUF afterwards.
- DMA: adjT ships f16 in group-major contiguous blocks, split across the
  sync + scalar hardware DGE queues (one queue ~140GB/s can't feed the
  16.8MB stream); in2/table/output DMAs ride the gpsimd software queue.
- v11: o1-of-previous-group emitted before o2-of-current (fills the DMA
  ramp window), adjT spread across sync/scalar/gpsimd queues 3 ways,
  6-group prefetch depth, final-rg output DMAs on the idle sync queue.
- v17: first adjacency group and in2 chunk split so the opening matmuls
  gate on half-size transfers; the final row-group's combine reads PSUM
  directly (ScalarE PSUM reads beat SBUF) and skips both evacuation
  copies -- nothing needs those banks afterwards.
- Host prep: e1=input1@a1, e2=input2@a2, their exp tables, f16 casts, and
  the adjacency transpose (same class of preprocessing as the original
  baseline's adj.T / f16 cast).
"""

import numpy as np
from contextlib import ExitStack

import concourse.bass as bass
import concourse.bacc as bacc
import concourse.tile as tile
from concourse import mybir
from concourse.bass_utils import run_bass_kernel_spmd

F32 = mybir.dt.float32
F16 = mybir.dt.float16

N_CORES = 8
N, M, D = 8192, 8192, 256
GAMMA = 0.1
P = 128
RB = 512
JC = M // P          # 64
JPG = 8
SHIFT = 4.0
AOP = mybir.AluOpType
AF = mybir.ActivationFunctionType

N_ACT = 4
NDVE = JPG - N_ACT


def build_kernel(nc, tc, ctx, rows):
    KR = rows // RB
    NC4 = RB // P
    NG = JC // JPG

    adjT_d = nc.dram_tensor("adjTs", [KR, NG, P, JPG, RB], F16,
                            kind="ExternalInput").ap()
    in2aug_d = nc.dram_tensor("in2aug", [P, JC, D + 1], F16,
                              kind="ExternalInput").ap()
    e1b_d = nc.dram_tensor("e1b", [P, rows], F32, kind="ExternalInput").ap()
    u16_d = nc.dram_tensor("u16", [P, 2, rows], F16, kind="ExternalInput").ap()
    vcol_d = nc.dram_tensor("vcol", [P, 3, JC], F32, kind="ExternalInput").ap()
    out_d = nc.dram_tensor("outs", [rows, D], F32, kind="ExternalOutput").ap()

    const_pool = ctx.enter_context(tc.tile_pool(name="const", bufs=1))
    adj_pool = ctx.enter_context(tc.tile_pool(name="adj", bufs=7))
    work_pool = ctx.enter_context(tc.tile_pool(name="work", bufs=3))
    quad_pool = ctx.enter_context(tc.tile_pool(name="quad", bufs=3))
    tail_pool = ctx.enter_context(tc.tile_pool(name="tail", bufs=2))
    out_pool = ctx.enter_context(tc.tile_pool(name="outp", bufs=3))
    ps_pool = ctx.enter_context(tc.tile_pool(name="ps", bufs=1, space="PSUM"))

    adjg = {}

    def load_group(rg, g):
        if g < NG:
            adjg[(rg, g)] = adj_pool.tile([P, JPG, RB], F16, tag="adjg",
                                          name=f"adjg_{rg}_{g}")
            # three-way queue split: even groups on sync, the first odd
            # groups on scalar (fast HW queue for the startup region), the
            # rest on the gpsimd software queue (high throughput once warm)
            idx = rg * NG + g
            if idx % 2 == 0:
                eng = nc.sync
            elif idx < 5:
                eng = nc.scalar
            else:
                eng = nc.gpsimd
            if idx == 0:
                # split the very first group so the jo<4 matmuls can start
                # after half the transfer
                eng.dma_start(out=adjg[(rg, g)][:, 0:JPG // 2, :],
                              in_=adjT_d[rg, g, :, 0:JPG // 2, :])
                eng.dma_start(out=adjg[(rg, g)][:, JPG // 2:, :],
                              in_=adjT_d[rg, g, :, JPG // 2:, :])
            else:
                eng.dma_start(out=adjg[(rg, g)][:], in_=adjT_d[rg, g])

    # first-needed data on the two HW queues, smallest-first
    in2aug = const_pool.tile([P, JC, D + 1], F16, tag="in2aug")
    nc.sync.dma_start(out=in2aug[:, 0:JPG // 2, :],
                      in_=in2aug_d[:, 0:JPG // 2, :])
    negc = const_pool.tile([P, 1], F32, tag="negc")
    nc.vector.memset(negc[:], -2.0 * SHIFT)
    e1b = const_pool.tile([P, rows], F32, tag="e1b")
    nc.scalar.dma_start(out=e1b[:, 0:RB], in_=e1b_d[:, 0:RB])
    nc.scalar.dma_start(out=e1b[:, RB:], in_=e1b_d[:, RB:])
    nc.sync.dma_start(out=in2aug[:, JPG // 2:JPG, :],
                      in_=in2aug_d[:, JPG // 2:JPG, :])
    vcol = const_pool.tile([P, 3, JC], F32, tag="vcol")
    nc.scalar.dma_start(out=vcol[:], in_=vcol_d)
    load_group(0, 0)   # sync
    u16 = const_pool.tile([P, 2, rows], F16, tag="u16")
    nc.scalar.dma_start(out=u16[:], in_=u16_d)
    e2s = vcol[:, 0, :]
    v1 = vcol[:, 1, :]
    v2 = vcol[:, 2, :]

    load_group(0, 1)   # scalar
    nc.sync.dma_start(out=in2aug[:, JPG:2 * JPG, :],
                      in_=in2aug_d[:, JPG:2 * JPG, :])
    load_group(0, 2)
    nc.sync.dma_start(out=in2aug[:, 2 * JPG:4 * JPG, :],
                      in_=in2aug_d[:, 2 * JPG:4 * JPG, :])
    load_group(0, 3)
    load_group(0, 4)
    # late in2aug chunks aren't latency-critical: gpsimd software queue
    nc.gpsimd.dma_start(out=in2aug[:, 4 * JPG:, :],
                        in_=in2aug_d[:, 4 * JPG:, :])
    load_group(0, 5)   # gpsimd

    for rg in range(KR):
        rs = slice(rg * RB, (rg + 1) * RB)
        o1 = ps_pool.tile([P, NC4, 512], F32, tag="o1", name=f"o1_{rg}")
        o2 = ps_pool.tile([P, NC4, 512], F32, tag="o2", name=f"o2_{rg}")
        # o1 matmuls run one jc-group behind o2 (stationaries then have a
        # full group period of slack -> PE never idles -> HAM stays warm)
        prev = None

        def emit_o1(stats, jg_of, stop_g):
            for jo in range(JPG):
                for c in range(NC4):
                    nc.tensor.matmul(o1[:, c, 0:D + 1],
                                     stats[jo][:, c * P:(c + 1) * P],
                                     in2aug[:, jg_of * JPG + jo, :],
                                     start=jg_of == 0 and jo == 0,
                                     stop=stop_g and jo == JPG - 1)

        for jg in range(NG):
            pre = jg + 6
            if pre < NG:
                load_group(rg, pre)
            elif rg + 1 < KR:
                load_group(rg + 1, pre - NG)
            ag = adjg.pop((rg, jg))
            first, last = jg == 0, jg == NG - 1
            # alternate 4/3 ACT-path tiles per group so ScalarE keeps slack;
            # gpsimd takes 3 mask-muls, VectorE the 4th (when present)
            nact = N_ACT - (jg % 2)
            ndve = JPG - nact

            # ---- elementwise ----
            attq = quad_pool.tile([P, ndve, RB], F16, tag="attq",
                                  name=f"attq_{rg}_{jg}")
            t1q = quad_pool.tile([P, ndve, RB], F16, tag="t1q",
                                 name=f"t1q_{rg}_{jg}")
            t2q = quad_pool.tile([P, ndve, RB], F16, tag="t2q",
                                 name=f"t2q_{rg}_{jg}")
            atta = []
            for jo in range(JPG):
                jc = jg * JPG + jo
                if jo < nact:  # ACT path; mask-mul on gpsimd (jo3: vector)
                    lr = work_pool.tile([P, RB], F32, tag="lr")
                    nc.scalar.activation(lr[:], e1b[:, rs], AF.Prelu,
                                         bias=e2s[:, jc:jc + 1], scale=1.0,
                                         alpha=0.2)
                    ex = work_pool.tile([P, RB], F16, tag="ex")
                    nc.scalar.activation(ex[:], lr[:], AF.Exp, bias=negc[:])
                    am = work_pool.tile([P, RB], F16, tag="attm", bufs=9,
                                        name=f"attm_{rg}_{jg}_{jo}")
                    eng = nc.vector if jo == 3 else nc.gpsimd
                    eng.tensor_mul(am[:], ex[:], ag[:, jo, :])
                    atta.append(am)
                else:  # DVE path
                    q = jo - nact
                    nc.vector.tensor_scalar_mul(t1q[:, q, :], u16[:, 0, rs],
                                                v1[:, jc:jc + 1])
                    nc.vector.tensor_scalar_mul(t2q[:, q, :], u16[:, 1, rs],
                                                v2[:, jc:jc + 1])
            nc.vector.tensor_tensor(out=t2q[:], in0=t1q[:], in1=t2q[:],
                                    op=AOP.max)
            nc.vector.tensor_mul(attq[:], t2q[:], ag[:, nact:, :])

            # ---- matmuls: o1 of the previous group first (its stationaries
            # are long ready), then o2 of this group (fresh DMA) ----
            if prev is not None:
                emit_o1(prev[0], prev[1], stop_g=False)
            for jo in range(JPG):
                for c in range(NC4):
                    nc.tensor.matmul(o2[:, c, 0:D + 1],
                                     ag[:, jo, c * P:(c + 1) * P],
                                     in2aug[:, jg * JPG + jo, :],
                                     start=first and jo == 0,
                                     stop=last and jo == JPG - 1)
            prev = ([(atta[jo] if jo < nact else attq[:, jo - nact, :])
                     for jo in range(JPG)], jg)

        # ---- tail. Interior rgs: evacuate PSUM with big ScalarE copies so
        # the banks free fast for the next rg, combine from SBUF. Final rg:
        # nobody needs the banks -- read PSUM directly (ScalarE PSUM reads
        # are faster than SBUF anyway) and skip both evacuation copies. ----
        sfx = f"_{rg}"
        final = rg + 1 >= KR
        if final:
            emit_o1(prev[0], prev[1], stop_g=True)
            s1, s2 = o1, o2
        else:
            o2c = tail_pool.tile([P, NC4, D + 1], F32, tag="o2c",
                                 name="o2c" + sfx)
            nc.scalar.copy(o2c[:], o2[:, :, 0:D + 1])
            emit_o1(prev[0], prev[1], stop_g=True)
            o1c = tail_pool.tile([P, NC4, D + 1], F32, tag="o1c",
                                 name="o1c" + sfx)
            nc.scalar.copy(o1c[:], o1[:, :, 0:D + 1])
            s1, s2 = o1c, o2c
        zr = tail_pool.tile([P, NC4], F32, tag="zr", name="zr" + sfx)
        nc.vector.tensor_scalar_add(zr[:], s1[:, :, D], 1e-30)
        rz = tail_pool.tile([P, NC4], F32, tag="rz", name="rz" + sfx)
        nc.vector.reciprocal(rz[:], zr[:])
        c1 = tail_pool.tile([P, NC4], F32, tag="c1", name="c1" + sfx)
        nc.vector.scalar_tensor_tensor(
            out=c1[:], in0=s2[:, :, D], scalar=GAMMA, in1=rz[:],
            op0=AOP.mult, op1=AOP.mult)
        for c in range(NC4):
            t = tail_pool.tile([P, D], F32, tag="t", name=f"t_{rg}_{c}")
            nc.scalar.mul(t[:], s1[:, c, 0:D], c1[:, c:c + 1])
            outt = out_pool.tile([P, D], F32, tag="outt", name=f"outt_{rg}_{c}")
            nc.vector.scalar_tensor_tensor(
                out=outt[:], in0=s2[:, c, 0:D], scalar=1.0 - GAMMA, in1=t[:],
                op0=AOP.mult, op1=AOP.add)
            # final rg: sync queue is idle by then and has lower latency
            oeng = nc.sync if final else nc.gpsimd
            oeng.dma_start(
                out=out_d[rg * RB + c * P: rg * RB + (c + 1) * P, :],
                in_=outt[:])


def build_nc(rows=N // N_CORES):
    nc = bacc.Bacc("TRN2", debug=False)
    with tile.TileContext(nc) as tc:
        with ExitStack() as ctx:
            build_kernel(nc, tc, ctx, rows)
    nc.compile()
    return nc


def kernel(input1, input2, adj, a1, a2, _trace=False):
    rows = input1.shape[0] // N_CORES
    KR, NG = rows // RB, JC // JPG
    nc = build_nc(rows=rows)

    e1 = (input1.astype(np.float64) @ a1.astype(np.float64)).ravel()
    e2 = (input2.astype(np.float64) @ a2.astype(np.float64)).ravel()

    vcol = np.stack([e2, np.exp(e2 - SHIFT), np.exp(0.2 * e2 - SHIFT)], 0)
    vcol = np.ascontiguousarray(
        vcol.astype(np.float32).reshape(3, JC, P).transpose(2, 0, 1))

    in2aug = np.ones((P, JC, D + 1), dtype=np.float16)
    in2aug[:, :, :D] = input2.reshape(JC, P, D).transpose(1, 0, 2)
    in2aug = np.ascontiguousarray(in2aug)

    in_maps = []
    for c in range(N_CORES):
        r0 = c * rows
        er = e1[r0:r0 + rows]
        u16 = np.stack([np.exp(er - SHIFT), np.exp(0.2 * er - SHIFT)], 0)
        u16 = np.ascontiguousarray(np.broadcast_to(
            u16.astype(np.float16), (P, 2, rows)))
        e1b = np.ascontiguousarray(np.broadcast_to(
            er.astype(np.float32), (P, rows)))
        ashard = adj[r0:r0 + rows].reshape(rows, JC, P)
        # [p, jc, r] -> [rg, g, p, jo, r]
        adjT = ashard.transpose(2, 1, 0).astype(np.float16)
        adjT = adjT.reshape(P, NG, JPG, KR, RB).transpose(3, 1, 0, 2, 4)
        in_maps.append({
            "adjTs": np.ascontiguousarray(adjT),
            "in2aug": in2aug,
            "e1b": e1b,
            "u16": u16,
            "vcol": vcol,
        })

    res = run_bass_kernel_spmd(nc, in_maps, list(range(N_CORES)), trace=_trace)
    out = np.concatenate([res.results[c]["outs"] for c in range(N_CORES)],
                         axis=0)
    if _trace:
        return out, res
    return out

